# revision 11
# baseline (speedup 1.0000x reference)
"""Trainium2 Bass kernel for the spatial-attention layer.

Math (reference):
    fp = input_h @ f            [B, N, D]   N = 64*64 = 4096, D = 64
    gp = x @ g                  [B, N, D]
    s  = gp @ fp^T              [B, N, N]
    beta = softmax(s, -1)
    o  = beta @ input_h         [B, N, C2]
    out = gamma * o + x

Distribution: 8 cores, core c handles batch b = c // 2 and query rows
[half*2048, (half+1)*2048) with half = c % 2. Each core sees the full
4096 keys of its batch.

Per-core strategy (v5):
  - PE warmup: ~100 dummy matmuls at t=0 trip the HAM activity window
    during the initial DMA wait so phase 1 runs at 2.4 GHz, not 1.2.
  - h tiles: f32 DMA -> fp16 cast (DVE/ACT alternating) -> fp16 PE
    transposes batched 4 per PSUM bank -> fpT projection; the fp8e4
    [h|h] pair tiles for the output matmul are fp16->fp8 casts.
    x tiles stay f32 (residual) and transpose in f32 on the PE; the
    evacuation copy casts to fp16.
  - f/g stationary tiles hold [chunk | chunk] duplicated columns so the
    projection outputs fpT/gpT land duplicated on partitions 0-63 and
    64-127 for free.
  - Phase 2 is emitted globally: scores + exp + denominators for ALL
    four query blocks first (the fp8e5 p matrix, 8 MB, stays resident),
    then all output matmuls.  The scheduler then always has score work
    to keep ACT's exp pipeline saturated while output matmuls fill the
    PE between score bursts.
  - Scores run as 2x row-tiled K=64 fp16 matmul pairs (tiles T0/T8 of
    the 64x128 PE mode), two key tiles concurrently into one
    [128, 1024] PSUM span, exp'd by a single ACT into a [128, 2, 512]
    fp8e5 pair tile (bias -4 keeps exp in range; cancels in softmax).
  - Output matmuls are fp8 Double-Row: one matmul per KEY-TILE PAIR
    per 128-query tile contracts 256 keys at once (p pair = lhsT,
    [h|h] pair = rhs e4m3, 512-channel moving, one PSUM bank).
  - Softmax denominators come from all-ones-stationary DoubleRow
    matmuls ([32, 512] accumulator, row 0 read), transposed to
    per-query layout with K=1 matmuls; normalization (gamma folded)
    is one reciprocal + two fused multiply-adds per output tile.
"""

import numpy as np

import concourse.bass as bass
import concourse.mybir as mybir
import concourse.tile as tile
from concourse import bacc
from concourse.bass_utils import run_bass_kernel_spmd

F32 = mybir.dt.float32
FP16 = mybir.dt.float16
FP8E4 = mybir.dt.float8e4
FP8E5 = mybir.dt.float8e5
MULT = mybir.AluOpType.mult
ADD = mybir.AluOpType.add
DR = mybir.MatmulPerfMode.DoubleRow

B, W, C, D = 4, 64, 512, 64
N = W * W                  # 4096 spatial positions (keys per batch)
NQ = N // 2                # 2048 queries per core
N_CORES = 8
MT = N // 128              # 32 key tiles
PAIRS = MT // 2            # 16 key-tile pairs
QB = 4                     # query blocks of 512
QT = NQ // 128             # 16 query tiles

EXP_FN = mybir.ActivationFunctionType.Exp


def build_nc():
    nc = bacc.Bacc(None)
    xh_d = nc.dram_tensor("xh", [NQ, C], F32, kind="ExternalInput")
    h_d = nc.dram_tensor("h", [N, C], F32, kind="ExternalInput")
    f_d = nc.dram_tensor("f", [C, D], F32, kind="ExternalInput")
    g_d = nc.dram_tensor("g", [C, D], F32, kind="ExternalInput")
    gamma_d = nc.dram_tensor("gamma", [1], F32, kind="ExternalInput")
    eye_d = nc.dram_tensor("eye", [128, 128], F32, kind="ExternalInput")
    out_d = nc.dram_tensor("out", [NQ, C], F32, kind="ExternalOutput")

    with tile.TileContext(nc) as tc:
        with (
            tc.tile_pool(name="consts", bufs=1) as consts,
            tc.tile_pool(name="hf_pool", bufs=8) as hf_pool,
            tc.tile_pool(name="h16_pool", bufs=8) as h16_pool,
            tc.tile_pool(name="h8_pool", bufs=PAIRS) as h8_pool,
            tc.tile_pool(name="stage", bufs=10) as stage_pool,
            tc.tile_pool(name="x_pool", bufs=QT) as x_pool,
            tc.tile_pool(name="p_pool", bufs=4 * PAIRS) as p_pool,
            tc.tile_pool(name="scales", bufs=8) as scales,
            tc.tile_pool(name="outp", bufs=4) as outp,
            tc.tile_pool(name="psA", bufs=2, space="PSUM") as psA,
            tc.tile_pool(name="psB", bufs=2, space="PSUM") as psB,
        ):
            # ---- PE warmup: >4us of matmuls during the DMA wait -------------
            dummy16 = consts.tile([128, 64], FP16)
            nc.vector.memset(dummy16, 0.0625)
            dum_ps = psA.tile([128, 1024], F32, tag="psA")
            for _ in range(72):
                nc.tensor.matmul(
                    dum_ps[0:64, 0:64], dummy16, dummy16,
                    start=True, stop=True,
                )

            # ---- constants -------------------------------------------------
            ident = consts.tile([128, 128], F32)
            nc.sync.dma_start(ident, eye_d[:, :])
            ident16 = consts.tile([128, 128], FP16)
            nc.vector.tensor_copy(ident16, ident)

            # bias for exp: -4 keeps exp(s) within fp8e5 range (s reaches
            # ~12.5, e5m2 max = e^10.96); the shift cancels in normalization.
            exp_bias = consts.tile([128, 1], F32)
            nc.vector.memset(exp_bias, -4.0)

            # DoubleRow needs >=32 output partitions: all-ones [128, 2, 32]
            # stationary; row 0 of the [32, 512] result is the denominator.
            ones8 = consts.tile([128, 2, 32], FP8E4)
            nc.vector.memset(ones8, 1.0)
            one32 = consts.tile([1, 1], F32)
            nc.vector.memset(one32, 1.0)

            gamma_sb = consts.tile([128, 1], F32)
            nc.sync.dma_start(
                gamma_sb,
                bass.AP(tensor=gamma_d, offset=0, ap=[[0, 128], [1, 1]]),
            )

            # f, g: [512, 64] -> per-k-chunk [128, 128] fp16 tiles with the
            # chunk duplicated on cols 0:64 and 64:128 so fpT/gpT come out
            # duplicated across both partition halves (for row tiling).
            fg_f32 = consts.tile([128, 2, 4, D], F32)
            nc.sync.dma_start(
                fg_f32[:, 0], f_d[:, :].rearrange("(k p) d -> p k d", p=128))
            nc.sync.dma_start(
                fg_f32[:, 1], g_d[:, :].rearrange("(k p) d -> p k d", p=128))
            f_tiles = [consts.tile([128, 2 * D], FP16, name=f"f16_{k}")
                       for k in range(4)]
            g_tiles = [consts.tile([128, 2 * D], FP16, name=f"g16_{k}")
                       for k in range(4)]
            for k in range(4):
                nc.vector.tensor_copy(f_tiles[k][:, 0:D], fg_f32[:, 0, k])
                nc.scalar.copy(f_tiles[k][:, D:2 * D], fg_f32[:, 0, k])
                nc.vector.tensor_copy(g_tiles[k][:, 0:D], fg_f32[:, 1, k])
                nc.scalar.copy(g_tiles[k][:, D:2 * D], fg_f32[:, 1, k])

            proj_f = [consts.tile([128, 512], FP16, name=f"projf_{i}")
                      for i in range(MT // 4)]
            proj_g = [consts.tile([128, 512], FP16, name=f"projg_{i}")
                      for i in range(QB)]

            # ---- phase 1b: load x (resident f32), build gpT = (x @ g)^T ----
            # x transposes in f32 (2 cyc/col on PE, but no fp16 cast needed);
            # the evacuation copy does the f32 -> fp16 conversion.
            x_sb = []

            def do_xgroup(ng):
                stg = [stage_pool.tile([128, 512], FP16, tag="stage",
                                       name=f"stg_x_{ng}_{k}")
                       for k in range(4)]
                xts = []
                for j in range(4):
                    n = ng * 4 + j
                    xt = x_pool.tile([128, C], F32, tag="x")
                    xts.append(xt)
                    x_sb.append(xt)
                    nc.sync.dma_start(xt, xh_d[n * 128:(n + 1) * 128, :])
                for k in range(4):
                    tpm = psB.tile([128, 512], F32, tag="psB2")
                    for j in range(4):
                        nc.tensor.transpose(
                            tpm[:, j * 128:(j + 1) * 128],
                            xts[j][:, k * 128:(k + 1) * 128], ident,
                        )
                    nc.vector.tensor_copy(stg[k], tpm)
                gpp = psB.tile([128, 512], F32, tag="psB")
                for k in range(4):
                    nc.tensor.matmul(
                        gpp, g_tiles[k], stg[k],
                        start=(k == 0), stop=(k == 3),
                    )
                nc.vector.tensor_copy(proj_g[ng][:, :], gpp)

            do_xgroup(0)

            # ---- phase 1a: load h; h16, h8 pairs, fpT = (h @ f)^T ----------
            h8_tiles = [h8_pool.tile([128, 2, 512], FP8E4, tag="h8",
                                     name=f"h8_{t}")
                        for t in range(PAIRS)]
            for mg in range(MT // 4):
                stg = [stage_pool.tile([128, 512], FP16, tag="stage",
                                       name=f"stg_h_{mg}_{k}")
                       for k in range(4)]
                h16s = []
                for j in range(4):
                    m = mg * 4 + j
                    hf = hf_pool.tile([128, C], F32, tag="hf")
                    nc.sync.dma_start(hf, h_d[m * 128:(m + 1) * 128, :])
                    h16 = h16_pool.tile([128, C], FP16, tag="h16")
                    h16s.append(h16)
                    nc.vector.tensor_copy(h16, hf)
                    h8v = h8_tiles[m // 2][:, m % 2]
                    if m < 16:
                        nc.vector.tensor_copy(h8v, h16)
                    else:
                        nc.gpsimd.tensor_copy(h8v, h16)
                for k in range(4):
                    tpm = psB.tile([128, 512], FP16, tag="psB2")
                    for j in range(4):
                        nc.tensor.transpose(
                            tpm[:, j * 128:(j + 1) * 128],
                            h16s[j][:, k * 128:(k + 1) * 128], ident16,
                        )
                    nc.vector.tensor_copy(stg[k], tpm)
                fpp = psB.tile([128, 512], F32, tag="psB")
                for k in range(4):
                    nc.tensor.matmul(
                        fpp, f_tiles[k], stg[k],
                        start=(k == 0), stop=(k == 3),
                    )
                nc.vector.tensor_copy(proj_f[mg][:, :], fpp)

            for ng in range(1, QT // 4):
                do_xgroup(ng)

            # ---- phase 2a: scores + exp + denominators for ALL blocks ------
            p_tiles = [[None] * PAIRS for _ in range(QB)]
            den_sbs = []
            den_ts = []
            for nb in range(QB):
                q0 = proj_g[nb][0:64, :]
                q1 = proj_g[nb][64:128, :]
                den_ps = psB.tile([32, 512], F32, tag="psB2")
                for t in range(PAIRS):
                    m0, m1 = 2 * t, 2 * t + 1
                    sps = psA.tile([128, 1024], F32, tag="psA")
                    nc.tensor.matmul(
                        sps[:, 0:512],
                        proj_f[m0 // 4][0:64, (m0 % 4) * 128:(m0 % 4 + 1) * 128],
                        q0, start=True, stop=True,
                    )
                    nc.tensor.matmul(
                        sps[:, 512:1024],
                        proj_f[m1 // 4][64:128, (m1 % 4) * 128:(m1 % 4 + 1) * 128],
                        q1, start=True, stop=True,
                    )
                    pt = p_pool.tile([128, 2, 512], FP8E5, tag="p")
                    p_tiles[nb][t] = pt
                    nc.scalar.activation(pt[:, :, :], sps[:, :], EXP_FN,
                                         bias=exp_bias)
                    # denominator: ones^T @ p pair -> [32, 512] accumulator
                    nc.tensor.matmul(
                        den_ps, ones8, pt[:, :, :],
                        start=(t == 0), stop=(t == PAIRS - 1),
                        perf_mode=DR,
                    )
                den_sb = scales.tile([1, 512], F32, tag="den",
                                     name=f"den_{nb}")
                nc.vector.tensor_copy(den_sb, den_ps[0:1, :])
                den_sbs.append(den_sb)
                den_t = psB.tile([128, 4], F32, tag="psB2")
                den_ts.append(den_t)
                for nt in range(4):
                    nc.tensor.matmul(
                        den_t[:, nt:nt + 1],
                        den_sb[0:1, nt * 128:(nt + 1) * 128],
                        one32, start=True, stop=True,
                    )
                sc4 = scales.tile([128, 4], F32, tag="scale",
                                  name=f"sc4_{nb}")
                den_sbs[nb] = sc4
                nc.vector.reciprocal(sc4, den_t)
                nc.vector.tensor_scalar_mul(sc4, sc4, gamma_sb)

            # ---- phase 2b: output matmuls for all blocks -------------------
            for nb in range(QB):
                sc4 = den_sbs[nb]
                for nt in range(4):
                    ops = psB.tile([128, 512], F32, tag="psB")
                    for t in range(PAIRS):
                        nc.tensor.matmul(
                            ops,
                            p_tiles[nb][t][:, :, nt * 128:(nt + 1) * 128],
                            h8_tiles[t][:, :, 0:512],
                            start=(t == 0), stop=(t == PAIRS - 1),
                            perf_mode=DR,
                        )
                    n_idx = nb * 4 + nt
                    xres = x_sb[n_idx]
                    out_sb = outp.tile([128, C], F32, tag="out")
                    nc.vector.scalar_tensor_tensor(
                        out_sb[:, 0:256], ops[:, 0:256], sc4[:, nt:nt + 1],
                        xres[:, 0:256], op0=MULT, op1=ADD)
                    nc.vector.scalar_tensor_tensor(
                        out_sb[:, 256:512], ops[:, 256:512], sc4[:, nt:nt + 1],
                        xres[:, 256:512], op0=MULT, op1=ADD)
                    nc.sync.dma_start(
                        out_d[n_idx * 128:(n_idx + 1) * 128, :], out_sb)

    nc.finalize()
    return nc


_NC_CACHE = None


def make_in_maps(x, input_h, f, g, gamma):
    x = np.asarray(x, dtype=np.float32)
    input_h = np.asarray(input_h, dtype=np.float32)
    f2 = np.ascontiguousarray(np.asarray(f, dtype=np.float32).reshape(C, D))
    g2 = np.ascontiguousarray(np.asarray(g, dtype=np.float32).reshape(C, D))
    gam = np.ascontiguousarray(np.asarray(gamma, dtype=np.float32).reshape(1))
    eye = np.eye(128, dtype=np.float32)

    x_flat = x.reshape(B, N, C)
    h_flat = input_h.reshape(B, N, C)

    in_maps = []
    for c in range(N_CORES):
        b, half = c // 2, c % 2
        in_maps.append({
            "xh": np.ascontiguousarray(x_flat[b, half * NQ:(half + 1) * NQ]),
            "h": np.ascontiguousarray(h_flat[b]),
            "f": f2,
            "g": g2,
            "gamma": gam,
            "eye": eye,
        })
    return in_maps


def kernel(x, input_h, f, g, gamma):
    global _NC_CACHE
    in_maps = make_in_maps(x, input_h, f, g, gamma)
    if _NC_CACHE is None:
        _NC_CACHE = build_nc()
    res = run_bass_kernel_spmd(_NC_CACHE, in_maps, core_ids=list(range(N_CORES)))

    out = np.empty((B, N, C), dtype=np.float32)
    for c in range(N_CORES):
        b, half = c // 2, c % 2
        out[b, half * NQ:(half + 1) * NQ] = res.results[c]["out"]
    return out.reshape(B, W, W, C)
